# revision 1
# baseline (speedup 1.0000x reference)
"""AttentionalJoin kernel for 8 Trainium2 NeuronCores.

Math: the reference builds full (M x M) self-attention over M = N+1 tokens
(CLS prepended) but returns only the CLS row of the projected output.  Only
the CLS query survives, so attention collapses to a softmax-weighted token
pooling:

    q       = Wq @ cls                       (per head h: q_h)
    score_t = scale * q_h . (Wk x_t)_h  =  x_t . R[:, h],   R = scale*Wk_h^T q_h
    p       = softmax over the M tokens (scores bounded ~[-6, 6]; no max-sub)
    pooled_h = sum_t p_t x_t                 (linearity: project AFTER pooling)
    out     = proj( concat_h Wv_h pooled_h ) + proj_b

Device pipeline, software-pipelined at depth 3 over 512-token chunks:
  T(i)   : 16 identity matmuls build X^T per c-chunk (x-block as fp16 FWL
           weights, N=128); PSUM->SBUF drains split 2:1 between DVE and ACT
  S(i-1) : scores TRANSPOSED: per t-block a 4-matmul chain over c-chunks
           with the X^T block as weights and R_q as the 8-col moving
           operand -> [t,h] in PSUM; one ACT exp yields E^T directly (no
           E-transpose pass) and one tiny matmul against a ones vector
           forms the per-chunk Z partition sums
  P(i-2) : 16 matmuls x_block.T @ E^T -> pooled^T [c128, h], one PSUM
           accumulation group per batch across all its chunks
x streams from HBM once in fp16 (rel err ~3e-4); the first chunks are
DMA'd in 128KB j-pieces issued alternately from SP/ACT HWDGE so the PE
starts early, and a burst of heater matmuls keeps the PE clock-gate
warming while they land.  The tiny tail (CLS term, 1/Z, head-mix, proj,
bias) runs on host.

Sharding: data-parallel over the batch dim, 2 batches per core.
"""

import numpy as np

H = 8
C = 512
HD = C // H
B = 16
N = 2048
NCORES = 8
BPC = B // NCORES          # batches per core
TOK = BPC * N              # tokens per core (4096)
NCHUNK = TOK // 512        # 512-token chunks per core (8; 4 per batch)
CPB = NCHUNK // BPC        # chunks per batch (4)
NSPLIT = 3                 # leading chunks DMA'd in j-pieces for early start
MAX_DRAIN_WAITS = 1        # this walrus rejects instructions w/ >1 sem wait

_cached = {}


def _patch_drain():
    """The container's walrus codegen rejects instructions carrying more
    than one sem wait ("Too many sync wait commands").  Split extra waits
    onto dedicated same-engine NOPs, which preserves semantics (engine
    queues are in-order)."""
    import concourse.tile as tile_mod
    from concourse import mybir
    from bass_rust import ScopedClock

    if getattr(tile_mod.TileContext, "_drain_patched", False):
        return

    orig_lower = tile_mod.TileContext._lower_ordered_insts

    def _lower_ordered_insts(self, ordered):
        nc = self.nc
        for bbname, insts in ordered.items():
            out = []
            for inst in insts:
                si = inst.sync_info
                if si is not None and si.on_wait and len(si.on_wait) > MAX_DRAIN_WAITS:
                    waits = list(si.on_wait)
                    extra, keep = waits[:-MAX_DRAIN_WAITS], waits[-MAX_DRAIN_WAITS:]
                    for w in extra:
                        nop = mybir.InstNoOp(
                            name=f"waitsplit-{nc.next_id()}",
                            engine=inst.engine,
                            ins=[],
                            outs=[],
                            bass_nofuse=True,
                            sync_info=mybir.SyncInfo(on_wait=[w], on_update=[]),
                            debug=inst.debug,
                        )
                        out.append(nop)
                    inst.sync_info = mybir.SyncInfo(
                        on_wait=keep, on_update=list(si.on_update)
                    )
                out.append(inst)
            ordered[bbname] = out
        return orig_lower(self, ordered)

    tile_mod.TileContext._lower_ordered_insts = _lower_ordered_insts

    def _drain_and_barrier(self, tick_clock, wait_clock):
        nc = self.nc
        probe = mybir.InstNoOp(
            name=f"drain-wait-probe-{nc.next_id()}",
            engine=mybir.EngineType.SP,
            ins=[],
            outs=[],
        )
        wait_clock.add_sem_waits(probe, ScopedClock({None: tick_clock.global_clock}))
        waits = list(probe.sync_info.on_wait) if probe.sync_info else []
        for i in range(0, len(waits), MAX_DRAIN_WAITS):
            chunk = waits[i : i + MAX_DRAIN_WAITS]
            nop = nc.sync.nop(nofuse=True, hint="drain_wait")
            nop.ins.sync_info = mybir.SyncInfo(on_wait=chunk, on_update=[])
        nc.sync.drain()

        nc.all_engine_barrier()
        popped = nc._tile_sem_poison_stack.pop()
        assert popped is self._sem_poison
        nc.clear_and_free_semaphores(list(self.sems.allocated().values()))
        nc.all_engine_barrier()

    tile_mod.TileContext._drain_and_barrier = _drain_and_barrier
    tile_mod.TileContext._drain_patched = True


def _build_module():
    import concourse.bass as bass
    import concourse.tile as tile
    from concourse import mybir
    from concourse.masks import make_identity

    _patch_drain()
    f16 = mybir.dt.float16
    f32 = mybir.dt.float32
    EXP = mybir.ActivationFunctionType.Exp

    nc = bass.Bass()
    x_in = nc.dram_tensor("x", [TOK, C], f16, kind="ExternalInput")
    r_in = nc.dram_tensor("r", [C, H], f16, kind="ExternalInput")
    # pooled^T per (batch, c-partition, c-chunk, head)
    s_out = nc.dram_tensor("s", [BPC, 128, 4, H], f32, kind="ExternalOutput")
    # Z partials per (batch, head, chunk-within-batch)
    z_out = nc.dram_tensor("z", [BPC, 32, CPB], f32, kind="ExternalOutput")

    x_whole = x_in.rearrange("(a j p) f -> a p j f", a=NCHUNK, j=4, p=128)
    x_piece = x_in.rearrange("(a j p) f -> a j p f", a=NCHUNK, j=4, p=128)
    r_src = r_in.rearrange("(q p) h -> p q h", p=128)

    with tile.TileContext(nc) as tc:
        with (
            tc.tile_pool(name="xpool", bufs=1) as xpool,
            tc.tile_pool(name="consts", bufs=1) as consts,
            tc.tile_pool(name="xtpool", bufs=2) as xtpool,
            tc.tile_pool(name="epool", bufs=2) as epool,
            tc.tile_pool(name="etpool", bufs=2) as etpool,
            tc.tile_pool(name="zpool", bufs=1) as zpool,
            tc.tile_pool(name="opool", bufs=1) as opool,
            tc.tile_pool(name="pt", bufs=4, space="PSUM") as pt_pool,
            tc.tile_pool(name="pst", bufs=2, space="PSUM") as pst_pool,
            tc.tile_pool(name="pz", bufs=1, space="PSUM") as pz_pool,
            tc.tile_pool(name="pp", bufs=1, space="PSUM") as pp_pool,
        ):
            r_sb = consts.tile([128, 4, H], f16)
            nc.sync.dma_start(out=r_sb, in_=r_src)

            # x loads: early chunks in 128KB j-pieces, alternating HWDGE
            # issue engines (SP / ACT) so the first tile lands ASAP
            x_sb = []
            ndma = 0

            def dma_eng():
                nonlocal ndma
                ndma += 1
                return nc.sync if ndma % 2 else nc.scalar

            for a in range(NCHUNK):
                t = xpool.tile([128, 4, C], f16, tag=f"x{a}", name=f"x{a}")
                x_sb.append(t)
            for a in range(NSPLIT):
                for j in range(4):
                    dma_eng().dma_start(out=x_sb[a][:, j, :], in_=x_piece[a, j])
            for a in range(NSPLIT, NCHUNK):
                dma_eng().dma_start(out=x_sb[a], in_=x_whole[a])

            ident = consts.tile([128, 128], f16)
            make_identity(nc, ident)

            # HAM heaters: keep the PE streaming while the first x pieces
            # land so the clock gate is at 8/8 when real work starts
            nheat = 0

            def heat(n):
                nonlocal nheat
                ht = pt_pool.tile([128, 128], f32, tag="pt", name=f"heat{nheat}")
                for k in range(n):
                    nc.tensor.matmul(ht, ident, ident, start=(k == 0), stop=(k == n - 1))
                    nheat += 1

            heat(14)

            ones = consts.tile([128, 1], f16)
            nc.vector.memset(ones, 1.0)
            # Z partials per (32 (j,h) rows, batch, chunk) in one PSUM bank
            pz = pz_pool.tile([32, BPC, CPB], f32, tag="pz", name="pz")
            # pooled^T accumulator: [c128, b, q, h] single PSUM bank
            pp = pp_pool.tile([128, BPC, 4, H], f32, tag="pp", name="pp")

            ncopy = 0

            def alt_copy(dst, src):
                nonlocal ncopy
                # PSUM->SBUF drains: DVE takes 2 of 3 (ACT also runs exp)
                if ncopy % 3 != 2:
                    nc.vector.tensor_copy(dst, src)
                else:
                    nc.scalar.copy(dst, src)
                ncopy += 1

            xts = {}
            es = {}
            ets = {}

            def stage_T(i):
                """transpose chunk i: per q, 4 identity MMs + PSUM drain."""
                xts[i] = []
                for q in range(4):
                    pt = pt_pool.tile([128, 512], f32, tag="pt", name=f"pt{i}_{q}")
                    for j in range(4):
                        # one accumulation group per pt bank: the 4 matmuls
                        # write disjoint column ranges (overwrite semantics),
                        # avoiding per-MM group micro-idles on the PE
                        nc.tensor.matmul(
                            pt[:, j * 128 : (j + 1) * 128],
                            x_sb[i][:, j, q * 128 : (q + 1) * 128],
                            ident,
                            start=(j == 0),
                            stop=(j == 3),
                        )
                    xt = xtpool.tile(
                        [128, 512], f16, tag=f"xt{q}", name=f"xt{i}_{q}"
                    )
                    alt_copy(xt, pt)
                    xts[i].append(xt)

            def stage_S(i):
                """scores chunk i, transposed form: per t-block j a 4-matmul
                chain over q with the X^T block as FWL weights and R_q as the
                8-col moving operand -> pst[:, j, :] = scores[t, h].  One ACT
                exp turns the whole tile into E^T directly (no E transpose
                pass), and one tiny matmul against a ones vector forms the
                per-chunk Z row-sums."""
                b, g = divmod(i, CPB)
                pst = pst_pool.tile([128, 4, H], f32, tag="pst", name=f"pst{i}")
                for j in range(4):
                    for q in range(4):
                        nc.tensor.matmul(
                            pst[:, j, :],
                            xts[i][q][:, j * 128 : (j + 1) * 128],
                            r_sb[:, q, :],
                            start=(q == 0),
                            stop=(q == 3),
                        )
                del xts[i]
                et = etpool.tile([128, 4, H], f16, tag="et", name=f"et{i}")
                nc.scalar.activation(out=et, in_=pst, func=EXP)
                es[i] = et
                nc.tensor.matmul(
                    pz[:, b, g : g + 1],
                    et,
                    ones,
                    start=True,
                    stop=True,
                )

            def stage_P(i):
                """pooled^T accumulation for chunk i (E^T comes from exp)."""
                b, g = divmod(i, CPB)
                et = es.pop(i)
                first = g == 0
                last = g == CPB - 1
                for q in range(4):
                    for j in range(4):
                        nc.tensor.matmul(
                            pp[:, b, q, :],
                            x_sb[i][:, j, q * 128 : (q + 1) * 128],
                            et[:, j, :],
                            start=(first and q == 0 and j == 0),
                            stop=(last and q == 3 and j == 3),
                        )

            def emit_out(b):
                so = opool.tile([128, 4, H], f32, tag=f"so{b}", name=f"so{b}")
                nc.vector.tensor_copy(so, pp[:, b])
                nc.scalar.dma_start(out=s_out[b], in_=so)
                zo = opool.tile([32, CPB], f32, tag=f"zo{b}", name=f"zo{b}")
                nc.vector.tensor_copy(zo, pz[:, b])
                nc.scalar.dma_start(out=z_out[b], in_=zo)

            for i in range(NCHUNK + 2):
                if i < NCHUNK:
                    stage_T(i)
                if 1 <= i <= NCHUNK:
                    stage_S(i - 1)
                if 2 <= i:
                    stage_P(i - 2)
                    if i - 2 == CPB - 1:
                        emit_out(0)
            emit_out(1)

    return nc


def _get_module():
    if "nc" not in _cached:
        _cached["nc"] = _build_module()
    return _cached["nc"]


def _host_prep(cls, qkv_w):
    scale = HD ** -0.5
    c = cls.reshape(C).astype(np.float64)
    Wq = qkv_w[:C].astype(np.float64)
    Wk = qkv_w[C : 2 * C].astype(np.float64)
    q = Wq @ c
    qh = q.reshape(H, HD)
    Wkh = Wk.reshape(H, HD, C)
    R = (scale * np.einsum("hdc,hd->ch", Wkh, qh)).astype(np.float16)
    k0 = Wk @ c
    score0 = scale * np.einsum("hd,hd->h", qh, k0.reshape(H, HD))
    e0 = np.exp(score0)
    return R, e0


def kernel(x, cls, qkv_w, proj_w, proj_b):
    from concourse.bass_utils import run_bass_kernel_spmd

    x = np.asarray(x, dtype=np.float32)
    cls = np.asarray(cls, dtype=np.float32)
    qkv_w = np.asarray(qkv_w, dtype=np.float32)
    proj_w = np.asarray(proj_w, dtype=np.float32)
    proj_b = np.asarray(proj_b, dtype=np.float32)

    R, e0 = _host_prep(cls, qkv_w)
    Wv = qkv_w[2 * C :]

    x16 = np.ascontiguousarray(x.reshape(B * N, C).astype(np.float16))
    nc = _get_module()
    in_maps = [
        {"x": x16[i * TOK : (i + 1) * TOK], "r": R}
        for i in range(NCORES)
    ]
    res = run_bass_kernel_spmd(nc, in_maps, list(range(NCORES)))
    _cached["last_results"] = res

    s_parts = []
    z_parts = []
    for i in range(NCORES):
        s_dev = res.results[i]["s"]          # [BPC, 128, 4, H]
        z_dev = res.results[i]["z"]          # [BPC, 32, CPB]
        s_parts.append(np.transpose(s_dev, (0, 3, 2, 1)).reshape(BPC, H, C))
        z_parts.append(z_dev.reshape(BPC, 4, H, CPB).sum(axis=(1, 3)))
    s_dev = np.concatenate(s_parts, axis=0)  # [B, H, C]
    z_dev = np.concatenate(z_parts, axis=0)  # [B, H]

    # add the CLS token's own contribution, normalize, head-mix + proj
    cf = cls.reshape(C)
    s_full = s_dev + (e0[:, None] * cf[None, :]).astype(np.float32)[None]
    z_full = z_dev + e0.astype(np.float32)[None]
    v = s_full / z_full[:, :, None]
    o = np.einsum("hdc,bhc->bhd", Wv.reshape(H, HD, C), v).reshape(B, C)
    y = o @ proj_w.T + proj_b
    return y.astype(np.float32)



# revision 7
# speedup vs baseline: 1.2722x; 1.2722x over previous
"""AttentionalJoin kernel for 8 Trainium2 NeuronCores.

Math: the reference builds full (M x M) self-attention over M = N+1 tokens
(CLS prepended) but returns only the CLS row of the projected output.  Only
the CLS query survives, so attention collapses to a softmax-weighted token
pooling:

    q       = Wq @ cls                       (per head h: q_h)
    score_t = scale * q_h . (Wk x_t)_h  =  x_t . R[:, h],   R = scale*Wk_h^T q_h
    p       = softmax over the M tokens
    pooled_h = sum_t p_t x_t                 (linearity: project AFTER pooling)
    out     = proj( concat_h Wv_h pooled_h ) + proj_b

Device design (v2): x streams from HBM ONCE in fp8e3 (e3m4) but in BOTH
layouts (natural [t, c] and transposed [c, t]) so that neither matmul needs
an on-chip transpose or a PSUM drain of x:

  scores : per t-block, 4 accumulating matmuls with the x^T block as
           fp8 stationary weights (fast-weight-load, 27 ns) and the 8-col
           R chunk as fp16 moving operand -> PSUM [t, 8]
  exp    : one ACT op per 4 t-blocks, exp(s - 3) -> E fp16 in SBUF
           (the -3 bias keeps e^s in fp16 range; it cancels in p = e/Z)
  pool   : per t-block, 4 matmuls with the natural x block as fp8
           stationary and E [t, 8] fp16 moving -> PSUM [c, 8], one
           accumulation group per (batch, c-chunk)
  Z      : per 4-block group, one matmul with E as stationary [t, 32]
           and a ones column moving -> PSUM [32, s]

PE cost ~ 264 small matmuls dominated by fp8 FWL weight loads (~27 ns each);
DMA cost ~ 4.3 MB -> the kernel is DMA-bound near the fp8 memory roofline.
e3m4 keeps end-to-end rel err ~1e-2 (e4m3 would be ~2e-2, at the gate).
The tiny tail (CLS term, 1/Z, head-mix, proj, bias) runs on host.

Sharding: data-parallel over the batch dim, 2 batches per core.
"""

import numpy as np

H = 8
C = 512
HD = C // H
B = 16
N = 2048
NCORES = 8
BPC = B // NCORES          # batches per core
TOK = BPC * N              # tokens per core (4096)
NTB = TOK // 128           # 128-token blocks per core (32)
NSG = NTB // 4             # 4-block score/pool groups (8; 4 per batch)
SGB = NSG // BPC           # groups per batch (4)
EXP_BIAS = -3.0
MAX_DRAIN_WAITS = 1        # this walrus rejects instructions w/ >1 sem wait

_cached = {}


def _patch_drain():
    """The container's walrus codegen rejects instructions carrying more
    than one sem wait ("Too many sync wait commands").  Split extra waits
    onto dedicated same-engine NOPs, which preserves semantics (engine
    queues are in-order)."""
    import concourse.tile as tile_mod
    from concourse import mybir
    from bass_rust import ScopedClock

    if getattr(tile_mod.TileContext, "_drain_patched", False):
        return

    orig_lower = tile_mod.TileContext._lower_ordered_insts

    def _lower_ordered_insts(self, ordered):
        nc = self.nc
        for bbname, insts in ordered.items():
            out = []
            for inst in insts:
                si = inst.sync_info
                if si is not None and si.on_wait and len(si.on_wait) > MAX_DRAIN_WAITS:
                    waits = list(si.on_wait)
                    extra, keep = waits[:-MAX_DRAIN_WAITS], waits[-MAX_DRAIN_WAITS:]
                    for w in extra:
                        nop = mybir.InstNoOp(
                            name=f"waitsplit-{nc.next_id()}",
                            engine=inst.engine,
                            ins=[],
                            outs=[],
                            bass_nofuse=True,
                            sync_info=mybir.SyncInfo(on_wait=[w], on_update=[]),
                            debug=inst.debug,
                        )
                        out.append(nop)
                    inst.sync_info = mybir.SyncInfo(
                        on_wait=keep, on_update=list(si.on_update)
                    )
                out.append(inst)
            ordered[bbname] = out
        return orig_lower(self, ordered)

    tile_mod.TileContext._lower_ordered_insts = _lower_ordered_insts

    def _drain_and_barrier(self, tick_clock, wait_clock):
        nc = self.nc
        probe = mybir.InstNoOp(
            name=f"drain-wait-probe-{nc.next_id()}",
            engine=mybir.EngineType.SP,
            ins=[],
            outs=[],
        )
        wait_clock.add_sem_waits(probe, ScopedClock({None: tick_clock.global_clock}))
        waits = list(probe.sync_info.on_wait) if probe.sync_info else []
        for i in range(0, len(waits), MAX_DRAIN_WAITS):
            chunk = waits[i : i + MAX_DRAIN_WAITS]
            nop = nc.sync.nop(nofuse=True, hint="drain_wait")
            nop.ins.sync_info = mybir.SyncInfo(on_wait=chunk, on_update=[])
        nc.sync.drain()

        nc.all_engine_barrier()
        popped = nc._tile_sem_poison_stack.pop()
        assert popped is self._sem_poison
        nc.clear_and_free_semaphores(list(self.sems.allocated().values()))
        nc.all_engine_barrier()

    tile_mod.TileContext._drain_and_barrier = _drain_and_barrier
    tile_mod.TileContext._drain_patched = True


def _build_module():
    import concourse.bass as bass
    import concourse.tile as tile
    from concourse import mybir
    from concourse.masks import make_identity

    _patch_drain()
    f8 = mybir.dt.float8e3
    f16 = mybir.dt.float16
    f32 = mybir.dt.float32
    EXP = mybir.ActivationFunctionType.Exp

    nc = bass.Bass()
    # x^T pieces: [s4][cq][128 c][1024 t]
    xt_in = nc.dram_tensor("xt", [4, 4, 128, 1024], f8, kind="ExternalInput")
    # x natural tiles: [s8][128 t][4 tb][512 c]
    xn_in = nc.dram_tensor("xn", [NSG, 128, 4, C], f8, kind="ExternalInput")
    # R: [cq][128 c][8 h] fp16
    r_in = nc.dram_tensor("r", [4, 128, H], f16, kind="ExternalInput")
    # pooled sums: [batch][128 c-within-chunk][cq][h]
    p_out = nc.dram_tensor("pool", [BPC, 128, 4, H], f32, kind="ExternalOutput")
    # Z partials: [(a, h) = 32][group]
    z_out = nc.dram_tensor("z", [32, NSG], f32, kind="ExternalOutput")

    with tile.TileContext(nc) as tc:
        with (
            tc.tile_pool(name="consts", bufs=1) as consts,
            tc.tile_pool(name="xtp", bufs=1) as xtp,
            tc.tile_pool(name="xnp", bufs=1) as xnp,
            tc.tile_pool(name="ep", bufs=1) as ep,
            tc.tile_pool(name="op", bufs=1) as op,
            tc.tile_pool(name="psc", bufs=2, space="PSUM") as psc_pool,
            tc.tile_pool(name="pht", bufs=1, space="PSUM") as pht_pool,
            tc.tile_pool(name="ppl", bufs=1, space="PSUM") as ppl_pool,
            tc.tile_pool(name="pzp", bufs=1, space="PSUM") as pzp_pool,
        ):
            ndma = 0

            def dma_eng():
                nonlocal ndma
                ndma += 1
                return nc.sync if ndma % 2 else nc.scalar

            xt_src = xt_in.rearrange("s q p f -> s q p f")
            xn_src = xn_in.rearrange("s p a f -> s p a f")

            r_sb = consts.tile([128, 4, H], f16)
            dma_eng().dma_start(out=r_sb, in_=r_in.rearrange("q p h -> p q h"))

            # x^T: one SBUF tile per c-chunk, filled in 4 column pieces so
            # scores can start as soon as the first 512 t-columns land
            xt_sb = [xtp.tile([128, TOK], f8, tag=f"xt{q}", name=f"xt{q}") for q in range(4)]
            for s4 in range(4):
                for q in range(4):
                    dma_eng().dma_start(
                        out=xt_sb[q][:, s4 * 1024 : (s4 + 1) * 1024],
                        in_=xt_src[s4, q],
                    )
            # x natural tiles (pooling stationaries), behind the x^T stream
            xn_sb = []
            for s in range(NSG):
                t = xnp.tile([128, 4, C], f8, tag=f"xn{s}", name=f"xn{s}")
                xn_sb.append(t)
                dma_eng().dma_start(out=t, in_=xn_src[s])

            ident = consts.tile([128, 128], f16)
            make_identity(nc, ident)
            bias_t = consts.tile([128, 1], f32)
            nc.vector.memset(bias_t, EXP_BIAS)
            ones = consts.tile([128, 1], f16)
            nc.vector.memset(ones, 1.0)

            # HAM heaters: keep the PE streaming while the first x pieces
            # land so the clock gate is at 8/8 when real work starts
            ht = pht_pool.tile([128, 128], f32, tag="heat", name="heat")
            NHEAT = 14
            for k in range(NHEAT):
                nc.tensor.matmul(ht, ident, ident, start=(k == 0), stop=(k == NHEAT - 1))

            # pooled accumulator [c-part, b, q, h]: one PSUM bank
            pp = ppl_pool.tile([128, BPC, 4, H], f32, tag="pp", name="pp")
            # Z partials [32, group]: one PSUM bank
            pz = pzp_pool.tile([32, NSG], f32, tag="pz", name="pz")

            es = {}

            def stage_scores(s):
                """scores for group s (4 t-blocks): 16 matmuls, x^T blocks
                stationary (fp8 FWL), R chunks moving; then one ACT exp."""
                ps = psc_pool.tile([128, 4, H], f32, tag="ps", name=f"ps{s}")
                for a in range(4):
                    tb = s * 4 + a
                    for q in range(4):
                        nc.tensor.matmul(
                            ps[:, a, :],
                            xt_sb[q][:, tb * 128 : (tb + 1) * 128],
                            r_sb[:, q, :],
                            start=(q == 0),
                            stop=(q == 3),
                        )
                et = ep.tile([128, 4, H], f16, tag=f"e{s}", name=f"e{s}")
                nc.scalar.activation(out=et, in_=ps, func=EXP, bias=bias_t, scale=1.0)
                es[s] = et

            def stage_pool(s):
                """pooled += x_block^T @ E_block for the 4 t-blocks of group
                s (natural x stationary, E moving); plus the group Z column."""
                b = s // SGB
                first = s % SGB == 0
                last = s % SGB == SGB - 1
                et = es[s]
                for a in range(4):
                    for q in range(4):
                        nc.tensor.matmul(
                            pp[:, b, q, :],
                            xn_sb[s][:, a, q * 128 : (q + 1) * 128],
                            et[:, a, :],
                            start=(first and a == 0 and q == 0),
                            stop=(last and a == 3 and q == 3),
                        )
                nc.tensor.matmul(
                    pz[:, s : s + 1], et, ones, start=True, stop=True
                )

            def emit_out(b):
                so = op.tile([128, 4, H], f32, tag=f"so{b}", name=f"so{b}")
                nc.vector.tensor_copy(so, pp[:, b])
                nc.scalar.dma_start(out=p_out[b], in_=so)

            for s in range(NSG):
                stage_scores(s)
            for s in range(NSG):
                stage_pool(s)
                if s == SGB - 1:
                    emit_out(0)
            emit_out(1)
            zo = op.tile([32, NSG], f32, tag="zo", name="zo")
            nc.vector.tensor_copy(zo, pz)
            nc.scalar.dma_start(out=z_out.rearrange("p f -> p f"), in_=zo)

    return nc


def _get_module():
    if "nc" not in _cached:
        _cached["nc"] = _build_module()
    return _cached["nc"]


def _host_prep(cls, qkv_w):
    scale = HD ** -0.5
    c = cls.reshape(C).astype(np.float64)
    Wq = qkv_w[:C].astype(np.float64)
    Wk = qkv_w[C : 2 * C].astype(np.float64)
    q = Wq @ c
    qh = q.reshape(H, HD)
    Wkh = Wk.reshape(H, HD, C)
    R = (scale * np.einsum("hdc,hd->ch", Wkh, qh)).astype(np.float16)
    k0 = Wk @ c
    score0 = scale * np.einsum("hd,hd->h", qh, k0.reshape(H, HD))
    e0 = np.exp(score0 + EXP_BIAS)
    return R, e0


def prepare_in_maps(x, cls, qkv_w):
    """Quantize x to e3m4 and build the per-core input maps (both layouts)."""
    import ml_dtypes

    R, e0 = _host_prep(cls, qkv_w)
    xq = np.ascontiguousarray(x.reshape(B * N, C)).astype(ml_dtypes.float8_e3m4)
    r_dev = np.ascontiguousarray(R.reshape(4, 128, H))
    in_maps = []
    for i in range(NCORES):
        xc = xq[i * TOK : (i + 1) * TOK]                       # [4096, 512]
        xn = np.ascontiguousarray(xc.reshape(NSG, 4, 128, C).swapaxes(1, 2))
        xt = np.ascontiguousarray(
            xc.T.reshape(4, 128, 4, 1024).transpose(2, 0, 1, 3)
        )  # [s4][cq][128 c][1024 t] from x^T [(q p) c-dims, (s4 j) t-dims]
        in_maps.append({"xt": xt, "xn": xn, "r": r_dev})
    return in_maps, e0


def kernel(x, cls, qkv_w, proj_w, proj_b):
    from concourse.bass_utils import run_bass_kernel_spmd

    x = np.asarray(x, dtype=np.float32)
    cls = np.asarray(cls, dtype=np.float32)
    qkv_w = np.asarray(qkv_w, dtype=np.float32)
    proj_w = np.asarray(proj_w, dtype=np.float32)
    proj_b = np.asarray(proj_b, dtype=np.float32)

    in_maps, e0 = prepare_in_maps(x, cls, qkv_w)
    Wv = qkv_w[2 * C :]

    nc = _get_module()
    res = run_bass_kernel_spmd(nc, in_maps, list(range(NCORES)))
    _cached["last_results"] = res

    s_parts = []
    z_parts = []
    for i in range(NCORES):
        p_dev = res.results[i]["pool"]       # [BPC, 128, 4, H]
        z_dev = res.results[i]["z"]          # [32, NSG] = [(a, h), group]
        # pooled[b, h, c] with c = q*128 + p
        s_parts.append(np.transpose(p_dev, (0, 3, 2, 1)).reshape(BPC, H, C))
        zg = z_dev.reshape(4, H, BPC, SGB)   # [(a), h, b, g]
        z_parts.append(zg.sum(axis=(0, 3)).T)  # [b, h]
    s_dev = np.concatenate(s_parts, axis=0)  # [B, H, C]
    z_dev = np.concatenate(z_parts, axis=0)  # [B, H]

    # add the CLS token's own contribution, normalize, head-mix + proj
    cf = cls.reshape(C).astype(np.float64)
    s_full = s_dev.astype(np.float64) + (e0[:, None] * cf[None, :])[None]
    z_full = z_dev.astype(np.float64) + e0[None]
    v = s_full / z_full[:, :, None]
    o = np.einsum("hdc,bhc->bhd", Wv.astype(np.float64).reshape(H, HD, C), v)
    y = o.reshape(B, C) @ proj_w.T.astype(np.float64) + proj_b.astype(np.float64)
    return y.astype(np.float32)


# revision 10
# speedup vs baseline: 1.4650x; 1.1516x over previous
"""AttentionalJoin kernel for 8 Trainium2 NeuronCores.

Math: the reference builds full (M x M) self-attention over M = N+1 tokens
(CLS prepended) but returns only the CLS row of the projected output.  Only
the CLS query survives, so attention collapses to a softmax-weighted token
pooling:

    q       = Wq @ cls                       (per head h: q_h)
    score_t = scale * q_h . (Wk x_t)_h  =  x_t . R[:, h],   R = scale*Wk_h^T q_h
    p       = softmax over the M tokens
    pooled_h = sum_t p_t x_t                 (linearity: project AFTER pooling)
    out     = proj( concat_h Wv_h pooled_h ) + proj_b

Device design (v2): x streams from HBM ONCE in fp8e3 (e3m4) but in BOTH
layouts (natural [t, c] and transposed [c, t]) so that neither matmul needs
an on-chip transpose or a PSUM drain of x:

  scores : per t-block, 4 accumulating matmuls with the x^T block as
           fp8 stationary weights (fast-weight-load, 27 ns) and the 8-col
           R chunk as fp16 moving operand -> PSUM [t, 8]
  exp    : one ACT op per 4 t-blocks, exp(s - 3) -> E fp16 in SBUF
           (the -3 bias keeps e^s in fp16 range; it cancels in p = e/Z)
  pool   : per t-block, 4 matmuls with the natural x block as fp8
           stationary and E [t, 8] fp16 moving -> PSUM [c, 8], one
           accumulation group per (batch, c-chunk)
  Z      : per 4-block group, one matmul with E as stationary [t, 32]
           and a ones column moving -> PSUM [32, s]

PE cost ~ 264 small matmuls dominated by fp8 FWL weight loads (~27 ns each);
DMA cost ~ 4.3 MB -> the kernel is DMA-bound near the fp8 memory roofline.
e3m4 keeps end-to-end rel err ~1e-2 (e4m3 would be ~2e-2, at the gate).
The tiny tail (CLS term, 1/Z, head-mix, proj, bias) runs on host.

Sharding: data-parallel over the batch dim, 2 batches per core.
"""

import numpy as np

H = 8
C = 512
HD = C // H
B = 16
N = 2048
NCORES = 8
BPC = B // NCORES          # batches per core
TOK = BPC * N              # tokens per core (4096)
NTB = TOK // 128           # 128-token blocks per core (32)
NSG = NTB // 4             # 4-block score/pool groups (8; 4 per batch)
SGB = NSG // BPC           # groups per batch (4)
EXP_BIAS = -3.0
MAX_DRAIN_WAITS = 1        # this walrus rejects instructions w/ >1 sem wait

_cached = {}


def _patch_drain():
    """The container's walrus codegen rejects instructions carrying more
    than one sem wait ("Too many sync wait commands").  Split extra waits
    onto dedicated same-engine NOPs, which preserves semantics (engine
    queues are in-order)."""
    import concourse.tile as tile_mod
    from concourse import mybir
    from bass_rust import ScopedClock

    if getattr(tile_mod.TileContext, "_drain_patched", False):
        return

    orig_lower = tile_mod.TileContext._lower_ordered_insts

    def _lower_ordered_insts(self, ordered):
        nc = self.nc
        for bbname, insts in ordered.items():
            out = []
            for inst in insts:
                si = inst.sync_info
                if si is not None and si.on_wait and len(si.on_wait) > MAX_DRAIN_WAITS:
                    waits = list(si.on_wait)
                    extra, keep = waits[:-MAX_DRAIN_WAITS], waits[-MAX_DRAIN_WAITS:]
                    for w in extra:
                        nop = mybir.InstNoOp(
                            name=f"waitsplit-{nc.next_id()}",
                            engine=inst.engine,
                            ins=[],
                            outs=[],
                            bass_nofuse=True,
                            sync_info=mybir.SyncInfo(on_wait=[w], on_update=[]),
                            debug=inst.debug,
                        )
                        out.append(nop)
                    inst.sync_info = mybir.SyncInfo(
                        on_wait=keep, on_update=list(si.on_update)
                    )
                out.append(inst)
            ordered[bbname] = out
        return orig_lower(self, ordered)

    tile_mod.TileContext._lower_ordered_insts = _lower_ordered_insts

    def _drain_and_barrier(self, tick_clock, wait_clock):
        nc = self.nc
        probe = mybir.InstNoOp(
            name=f"drain-wait-probe-{nc.next_id()}",
            engine=mybir.EngineType.SP,
            ins=[],
            outs=[],
        )
        wait_clock.add_sem_waits(probe, ScopedClock({None: tick_clock.global_clock}))
        waits = list(probe.sync_info.on_wait) if probe.sync_info else []
        for i in range(0, len(waits), MAX_DRAIN_WAITS):
            chunk = waits[i : i + MAX_DRAIN_WAITS]
            nop = nc.sync.nop(nofuse=True, hint="drain_wait")
            nop.ins.sync_info = mybir.SyncInfo(on_wait=chunk, on_update=[])
        nc.sync.drain()

        nc.all_engine_barrier()
        popped = nc._tile_sem_poison_stack.pop()
        assert popped is self._sem_poison
        nc.clear_and_free_semaphores(list(self.sems.allocated().values()))
        nc.all_engine_barrier()

    tile_mod.TileContext._drain_and_barrier = _drain_and_barrier
    tile_mod.TileContext._drain_patched = True


def _build_module():
    import concourse.bass as bass
    import concourse.tile as tile
    from concourse import mybir
    from concourse.masks import make_identity

    _patch_drain()
    f8 = mybir.dt.float8e3
    f16 = mybir.dt.float16
    f32 = mybir.dt.float32
    EXP = mybir.ActivationFunctionType.Exp

    nc = bass.Bass()
    # x^T pieces: [s4][cq][128 c][1024 t]  (one DMA per s4 super)
    xt_in = nc.dram_tensor("xt", [4, 4, 128, 1024], f8, kind="ExternalInput")
    # x natural tiles: [s8][128 t][4 tb][512 c]  (one DMA per 2 tiles)
    xn_in = nc.dram_tensor("xn", [NSG, 128, 4, C], f8, kind="ExternalInput")
    # R: [cq][128 c][8 h] fp16
    r_in = nc.dram_tensor("r", [4, 128, H], f16, kind="ExternalInput")
    # pooled sums: [batch][128 c-within-chunk][cq][h]
    p_out = nc.dram_tensor("pool", [BPC, 128, 4, H], f32, kind="ExternalOutput")
    # Z partials: [(a, h) = 32][group]
    z_out = nc.dram_tensor("z", [32, NSG], f32, kind="ExternalOutput")

    with tile.TileContext(nc) as tc:
        with (
            tc.tile_pool(name="consts", bufs=1) as consts,
            tc.tile_pool(name="xtp", bufs=1) as xtp,
            tc.tile_pool(name="xnp", bufs=1) as xnp,
            tc.tile_pool(name="ep", bufs=1) as ep,
            tc.tile_pool(name="op", bufs=1) as op,
            tc.tile_pool(name="psc", bufs=2, space="PSUM") as psc_pool,
            tc.tile_pool(name="pht", bufs=1, space="PSUM") as pht_pool,
            tc.tile_pool(name="ppl", bufs=1, space="PSUM") as ppl_pool,
            tc.tile_pool(name="pzp", bufs=1, space="PSUM") as pzp_pool,
        ):
            xt_src = xt_in.rearrange("s q p f -> s p q f")
            xn_src = xn_in.rearrange("(u v) p a f -> u p v a f", v=2)

            # R rides on the ACT ring (ahead of the exps); everything else
            # on SP so the ~600ns-per-doorbell HWDGE serialization never
            # blocks the exp stream
            r_sb = consts.tile([128, 4, H], f16)
            nc.scalar.dma_start(out=r_sb, in_=r_in.rearrange("q p h -> p q h"))

            # x^T: a single [c-part, cq, t] tile; one 512KB DMA per t-super
            # so scores unlock progressively while the stream flows
            xt_sb = xtp.tile([128, 4, TOK], f8, tag="xt", name="xt")
            for s4 in range(4):
                nc.sync.dma_start(
                    out=xt_sb[:, :, s4 * 1024 : (s4 + 1) * 1024],
                    in_=xt_src[s4],
                )
            # x natural tiles (pooling stationaries): 512KB DMAs, 2 tiles each
            xn_sb = xnp.tile([128, NSG, 4, C], f8, tag="xn", name="xn")
            for u in range(4):
                nc.sync.dma_start(out=xn_sb[:, 2 * u : 2 * u + 2], in_=xn_src[u])

            ident = consts.tile([128, 128], f16)
            make_identity(nc, ident)
            bias_t = consts.tile([128, 1], f32)
            nc.vector.memset(bias_t, EXP_BIAS)
            ones = consts.tile([128, 1], f16)
            nc.vector.memset(ones, 1.0)

            # ACT warm-up: triggers the 1.3µs exp LUT table load while the
            # x stream is still in flight
            warm = consts.tile([128, 1], f16)
            nc.scalar.activation(out=warm, in_=bias_t, func=EXP, bias=bias_t, scale=1.0)

            # HAM heaters: keep the PE streaming while the first x pieces
            # land so the clock gate is at 8/8 when real work starts
            ht = pht_pool.tile([128, 128], f32, tag="heat", name="heat")
            NHEAT = 14
            for k in range(NHEAT):
                nc.tensor.matmul(ht, ident, ident, start=(k == 0), stop=(k == NHEAT - 1))

            # pooled accumulator [c-part, b, q, h]: one PSUM bank
            pp = ppl_pool.tile([128, BPC, 4, H], f32, tag="pp", name="pp")
            # Z partials [32, group]: one PSUM bank
            pz = pzp_pool.tile([32, NSG], f32, tag="pz", name="pz")

            es = {}

            def stage_scores(s):
                """scores for group s (4 t-blocks): 16 matmuls, x^T blocks
                stationary (fp8 FWL), R chunks moving; then one ACT exp."""
                ps = psc_pool.tile([128, 4, H], f32, tag="ps", name=f"ps{s}")
                for a in range(4):
                    tb = s * 4 + a
                    for q in range(4):
                        nc.tensor.matmul(
                            ps[:, a, :],
                            xt_sb[:, q, tb * 128 : (tb + 1) * 128],
                            r_sb[:, q, :],
                            start=(q == 0),
                            stop=(q == 3),
                        )
                et = ep.tile([128, 4, H], f16, tag=f"e{s}", name=f"e{s}")
                nc.scalar.activation(out=et, in_=ps, func=EXP, bias=bias_t, scale=1.0)
                es[s] = et

            def stage_pool(s):
                """pooled += x_block^T @ E_block for the 4 t-blocks of group
                s (natural x stationary, E moving); plus the group Z column."""
                b = s // SGB
                first = s % SGB == 0
                last = s % SGB == SGB - 1
                et = es[s]
                for a in range(4):
                    for q in range(4):
                        nc.tensor.matmul(
                            pp[:, b, q, :],
                            xn_sb[:, s, a, q * 128 : (q + 1) * 128],
                            et[:, a, :],
                            start=(first and a == 0 and q == 0),
                            stop=(last and a == 3 and q == 3),
                        )
                nc.tensor.matmul(
                    pz[:, s : s + 1], et, ones, start=True, stop=True
                )

            def emit_out(b):
                so = op.tile([128, 4, H], f32, tag=f"so{b}", name=f"so{b}")
                nc.vector.tensor_copy(so, pp[:, b])
                nc.sync.dma_start(out=p_out[b], in_=so)

            # emission order matches data arrival: xt supers 0-3 unlock
            # score pairs, xn halves unlock pool pairs
            for s in (0, 1, 2, 3, 4, 5):
                stage_scores(s)
            stage_pool(0)
            stage_pool(1)
            stage_scores(6)
            stage_scores(7)
            for s in (2, 3):
                stage_pool(s)
            emit_out(0)
            for s in (4, 5, 6, 7):
                stage_pool(s)
            emit_out(1)
            zo = op.tile([32, NSG], f32, tag="zo", name="zo")
            nc.vector.tensor_copy(zo, pz)
            nc.sync.dma_start(out=z_out.rearrange("p f -> p f"), in_=zo)

    return nc


def _get_module():
    if "nc" not in _cached:
        _cached["nc"] = _build_module()
    return _cached["nc"]


def _host_prep(cls, qkv_w):
    scale = HD ** -0.5
    c = cls.reshape(C).astype(np.float64)
    Wq = qkv_w[:C].astype(np.float64)
    Wk = qkv_w[C : 2 * C].astype(np.float64)
    q = Wq @ c
    qh = q.reshape(H, HD)
    Wkh = Wk.reshape(H, HD, C)
    R = (scale * np.einsum("hdc,hd->ch", Wkh, qh)).astype(np.float16)
    k0 = Wk @ c
    score0 = scale * np.einsum("hd,hd->h", qh, k0.reshape(H, HD))
    e0 = np.exp(score0 + EXP_BIAS)
    return R, e0


def prepare_in_maps(x, cls, qkv_w):
    """Quantize x to e3m4 and build the per-core input maps (both layouts)."""
    import ml_dtypes

    R, e0 = _host_prep(cls, qkv_w)
    xq = np.ascontiguousarray(x.reshape(B * N, C)).astype(ml_dtypes.float8_e3m4)
    r_dev = np.ascontiguousarray(R.reshape(4, 128, H))
    in_maps = []
    for i in range(NCORES):
        xc = xq[i * TOK : (i + 1) * TOK]                       # [4096, 512]
        xn = np.ascontiguousarray(xc.reshape(NSG, 4, 128, C).swapaxes(1, 2))
        xt = np.ascontiguousarray(
            xc.T.reshape(4, 128, 4, 1024).transpose(2, 0, 1, 3)
        )  # [s4][cq][128 c][1024 t] from x^T [(q p) c-dims, (s4 j) t-dims]
        in_maps.append({"xt": xt, "xn": xn, "r": r_dev})
    return in_maps, e0


def kernel(x, cls, qkv_w, proj_w, proj_b):
    from concourse.bass_utils import run_bass_kernel_spmd

    x = np.asarray(x, dtype=np.float32)
    cls = np.asarray(cls, dtype=np.float32)
    qkv_w = np.asarray(qkv_w, dtype=np.float32)
    proj_w = np.asarray(proj_w, dtype=np.float32)
    proj_b = np.asarray(proj_b, dtype=np.float32)

    in_maps, e0 = prepare_in_maps(x, cls, qkv_w)
    Wv = qkv_w[2 * C :]

    nc = _get_module()
    res = run_bass_kernel_spmd(nc, in_maps, list(range(NCORES)))
    _cached["last_results"] = res

    s_parts = []
    z_parts = []
    for i in range(NCORES):
        p_dev = res.results[i]["pool"]       # [BPC, 128, 4, H]
        z_dev = res.results[i]["z"]          # [32, NSG] = [(a, h), group]
        # pooled[b, h, c] with c = q*128 + p
        s_parts.append(np.transpose(p_dev, (0, 3, 2, 1)).reshape(BPC, H, C))
        zg = z_dev.reshape(4, H, BPC, SGB)   # [(a), h, b, g]
        z_parts.append(zg.sum(axis=(0, 3)).T)  # [b, h]
    s_dev = np.concatenate(s_parts, axis=0)  # [B, H, C]
    z_dev = np.concatenate(z_parts, axis=0)  # [B, H]

    # add the CLS token's own contribution, normalize, head-mix + proj
    cf = cls.reshape(C).astype(np.float64)
    s_full = s_dev.astype(np.float64) + (e0[:, None] * cf[None, :])[None]
    z_full = z_dev.astype(np.float64) + e0[None]
    v = s_full / z_full[:, :, None]
    o = np.einsum("hdc,bhc->bhd", Wv.astype(np.float64).reshape(H, HD, C), v)
    y = o.reshape(B, C) @ proj_w.T.astype(np.float64) + proj_b.astype(np.float64)
    return y.astype(np.float32)
